# revision 15
# baseline (speedup 1.0000x reference)
"""Trainium2 Bass kernel for nn_Block_local (dual global/banded-local attention block).

Sharding: pure data-parallel - one batch element per NeuronCore (B=8, 8 cores).

Strategy:
  - Host-side marshaling: x transposed to feature-major bf16; weights quantized
    to fp8e4 in DoubleRow-friendly [128, kc, out] layouts (contraction dims
    zero-padded to multiples of 256).
  - Big matmuls (qkv both branches, AV, global proj, fc1, fc2) run fp8
    DoubleRow (0.5 cycles/row) - 4x PE throughput vs f32r.
  - MLP accuracy recovered via fp8 residual compensation: h is split into
    h8 + hlo8 (exact two-term fp8 representation of 4*LN2(x)), and W1/W2 each
    ship as a (hi, lo) fp8 pair with lo = quantization residual.  The extra
    terms are more cheap DoubleRow matmuls into the same PSUM accumulator.
  - Residual spine in bf16 feature-major (xt); output written feature-major
    bf16, transposed back on host.
  - Local (banded ks=3) attention on DVE in bf16, interleaved into the global
    scores phase so DVE crunches band math while PE does scores/AV.
"""
import os
import numpy as np
import ml_dtypes

import concourse.bass as bass
import concourse.bacc as bacc
import concourse.mybir as mybir
import concourse.tile as tile
from concourse.bass_utils import run_bass_kernel_spmd
from concourse.masks import make_identity
from contextlib import ExitStack

F32 = mybir.dt.float32
BF16 = mybir.dt.bfloat16
FP8 = mybir.dt.float8e4
AF = mybir.ActivationFunctionType
ALU = mybir.AluOpType
AX = mybir.AxisListType
DR = mybir.MatmulPerfMode.DoubleRow

NP_BF16 = ml_dtypes.bfloat16
NP_FP8 = ml_dtypes.float8_e4m3

B, N, C = 8, 1024, 768
GD = 384          # per-branch feature dim
H, D = 6, 64      # heads, head dim
SCALE = D ** -0.5
HID = 3072
EPS = 1e-6
NH = 2            # token n-halves of 512
NHW = N // NH     # 512
MC = N // 128     # 8 token chunks
CC = C // 128     # 6 feature chunks
GC = GD // 128    # 3 feature chunks per branch
JC = HID // 128   # 24 hidden chunks
WS = 64.0         # attention weight quantization scale
WS1 = 256.0       # fc1 weight scale (h8 carries x4 -> PSUM = 1024 * true)
WS2 = 1024.0      # fc2 weight scale


def _build(flags):
    nc = bacc.Bacc("TRN2", target_bir_lowering=False, debug=False)

    xT_d = nc.dram_tensor("xT", (C, N), BF16, kind="ExternalInput")
    wqk_d = nc.dram_tensor("wqk8", (512, 2 * GD), FP8, kind="ExternalInput")
    wv_d = nc.dram_tensor("wv8", (512, GD), FP8, kind="ExternalInput")
    wl_d = nc.dram_tensor("wl8", (512, 3 * GD), FP8, kind="ExternalInput")
    wpg_d = nc.dram_tensor("wpg8", (512, GD), FP8, kind="ExternalInput")
    wpl_d = nc.dram_tensor("wpl8", (512, GD), FP8, kind="ExternalInput")
    w1h_d = nc.dram_tensor("w1h8", (C, HID), FP8, kind="ExternalInput")
    w1l_d = nc.dram_tensor("w1l8", (C, HID), FP8, kind="ExternalInput")
    w2h_d = nc.dram_tensor("w2h8", (HID, C), FP8, kind="ExternalInput")
    w2l_d = nc.dram_tensor("w2l8", (HID, C), FP8, kind="ExternalInput")
    fc1b_d = nc.dram_tensor("fc1bias", (HID,), F32, kind="ExternalInput")
    out_d = nc.dram_tensor("outT", (C, N), BF16, kind="ExternalOutput")
    gpb_d = nc.dram_tensor("gpb", (GD,), F32, kind="ExternalInput") if flags["bias_gproj"] else None
    lpb_d = nc.dram_tensor("lpb", (GD,), F32, kind="ExternalInput") if flags["bias_lproj"] else None
    fc2b_d = nc.dram_tensor("fc2bias", (C,), F32, kind="ExternalInput") if flags["bias_fc2"] else None
    g1_d = nc.dram_tensor("ln1gb", (2, GD), F32, kind="ExternalInput") if flags["gb1g"] else None
    l1_d = nc.dram_tensor("ln1lgb", (2, GD), F32, kind="ExternalInput") if flags["gb1l"] else None

    with tile.TileContext(nc) as tc, ExitStack() as top:
        consts = top.enter_context(tc.tile_pool(name="consts", bufs=1))
        core = top.enter_context(tc.tile_pool(name="core", bufs=1))
        wpool = top.enter_context(tc.tile_pool(name="wpool", bufs=1))

        ident8 = consts.tile([128, 128], FP8, tag="ident8")
        make_identity(nc, ident8)
        ones_c = consts.tile([128, 1], BF16, tag="ones_c")
        nc.vector.memset(ones_c, 1.0)
        onebc = consts.tile([1, 128], BF16, tag="onebc")
        nc.vector.memset(onebc, 1.0)
        c64 = consts.tile([1, 64], BF16, tag="c64")
        nc.vector.memset(c64, 1.0 / WS)
        eps_t = consts.tile([1, 1], F32, tag="eps")
        nc.vector.memset(eps_t, EPS)
        eps16_t = consts.tile([1, 1], F32, tag="eps16")
        nc.vector.memset(eps16_t, EPS / 16.0)
        zeros16 = consts.tile([1, 3 * GD], BF16, tag="zeros16")
        nc.vector.memset(zeros16, 0.0)

        # ---- load inputs ----
        xt = core.tile([128, CC, N], BF16, tag="xt")
        nc.sync.dma_start(xt, xT_d.rearrange("(c p) n -> p c n", p=128))
        wqk = wpool.tile([128, 4, 2 * GD], FP8, tag="wqk")
        nc.sync.dma_start(wqk, wqk_d.rearrange("(c p) o -> p c o", p=128))
        wv = wpool.tile([128, 4, GD], FP8, tag="wv")
        nc.sync.dma_start(wv, wv_d.rearrange("(c p) o -> p c o", p=128))
        wl = wpool.tile([128, 4, 3 * GD], FP8, tag="wl")
        nc.sync.dma_start(wl, wl_d.rearrange("(c p) o -> p c o", p=128))
        wpg = wpool.tile([128, 4, GD], FP8, tag="wpg")
        nc.sync.dma_start(wpg, wpg_d.rearrange("(c p) o -> p c o", p=128))
        wpl = wpool.tile([128, 4, GD], FP8, tag="wpl")
        nc.sync.dma_start(wpl, wpl_d.rearrange("(c p) o -> p c o", p=128))
        fc1b = wpool.tile([128, JC], F32, tag="fc1b")
        nc.sync.dma_start(fc1b, fc1b_d.rearrange("(c p) -> p c", p=128))

        def load_vec(dram, n_elems, tag):
            t = consts.tile([128, n_elems // 128], F32, tag=tag)
            nc.sync.dma_start(t, dram.rearrange("(c p) -> p c", p=128))
            return t

        gpb = load_vec(gpb_d, GD, "gpb") if gpb_d is not None else None
        lpb = load_vec(lpb_d, GD, "lpb") if lpb_d is not None else None
        fc2b = load_vec(fc2b_d, C, "fc2b") if fc2b_d is not None else None
        g1gb = None
        if g1_d is not None:
            g1gb = consts.tile([128, 2, GC], F32, tag="g1gb")
            nc.sync.dma_start(g1gb, g1_d.rearrange("t (c p) -> p t c", p=128))
        l1gb = None
        if l1_d is not None:
            l1gb = consts.tile([128, 2, GC], F32, tag="l1gb")
            nc.sync.dma_start(l1gb, l1_d.rearrange("t (c p) -> p t c", p=128))

        attn_scope = tc.tile_pool(name="attn", bufs=1)
        attn = attn_scope.__enter__()
        x8g = attn.tile([128, 4, N], FP8, tag="x8g")
        x8l = attn.tile([128, 4, N], FP8, tag="x8l")
        nc.gpsimd.memset(x8g[:, GC, :], 0.0)
        nc.gpsimd.memset(x8l[:, GC, :], 0.0)

        # ---------------- feature-major LayerNorm ----------------
        def ln_feat(lo, hi, dst, nh, st_p, bc_p, sm_p, sq_p, gb, post4):
            """dst[:, c-lo, ns] = LN(xt rows [lo*128, hi*128)), token half nh.
            post4: output scaled by 4 (folded into rstd via Sqrt scale)."""
            nch = hi - lo
            inv = 1.0 / (nch * 128)
            ns = slice(nh * NHW, (nh + 1) * NHW)
            sq = sq_p.tile([128, nch, NHW], BF16, tag="sq")
            nc.vector.tensor_tensor(sq, xt[:, lo:hi, ns], xt[:, lo:hi, ns], ALU.mult)
            st = st_p.tile([1, 2 * NHW], F32, tag="st")
            for i, c in enumerate(range(lo, hi)):
                nc.tensor.matmul(st[:, 0:NHW], ones_c, xt[:, c, ns],
                                 start=(i == 0), stop=(i == nch - 1))
            for i in range(nch):
                nc.tensor.matmul(st[:, NHW:2 * NHW], ones_c, sq[:, i, :],
                                 start=(i == 0), stop=(i == nch - 1))
            me = sm_p.tile([1, 2 * NHW], F32, tag="me")
            nc.vector.tensor_scalar_mul(me, st, inv)   # [mean | e2]
            mean, e2 = me[:, 0:NHW], me[:, NHW:2 * NHW]
            m2 = sm_p.tile([1, NHW], F32, tag="m2")
            nc.scalar.activation(m2, mean, AF.Square)
            var = sm_p.tile([1, NHW], F32, tag="var")
            nc.vector.tensor_tensor(var, e2, m2, ALU.subtract)
            sr = sm_p.tile([1, NHW], F32, tag="sr")
            if post4:
                nc.scalar.activation(sr, var, AF.Sqrt, bias=eps16_t, scale=1.0 / 16.0)
            else:
                nc.scalar.activation(sr, var, AF.Sqrt, bias=eps_t, scale=1.0)
            rstd = sm_p.tile([1, NHW], BF16, tag="rstd")
            with nc.allow_low_precision(reason="bf16 matmul operand"):
                nc.vector.reciprocal(rstd, sr)
            mrb = sm_p.tile([1, NHW], BF16, tag="mrb")
            nc.vector.tensor_tensor(mrb, rstd, mean, ALU.mult)
            rb_b = bc_p.tile([128, 2 * NHW], F32, tag="rb_b")
            nc.tensor.matmul(rb_b[:, 0:NHW], onebc, rstd, start=True, stop=True)
            nc.tensor.matmul(rb_b[:, NHW:2 * NHW], onebc, mrb, start=True, stop=True)
            for c in range(lo, hi):
                t16 = sq_p.tile([128, NHW], BF16, tag="t16")
                nc.vector.tensor_tensor(t16, xt[:, c, ns], rb_b[:, 0:NHW], ALU.mult)
                if gb is not None:
                    t2 = sq_p.tile([128, NHW], F32, tag="t2f")
                    nc.vector.tensor_tensor(t2, t16, rb_b[:, NHW:2 * NHW], ALU.subtract)
                    nc.vector.tensor_scalar(dst[:, c - lo, ns], t2,
                                            gb[:, 0, c - lo:c - lo + 1],
                                            gb[:, 1, c - lo:c - lo + 1],
                                            ALU.mult, ALU.add)
                else:
                    nc.vector.tensor_tensor(dst[:, c - lo, ns], t16,
                                            rb_b[:, NHW:2 * NHW], ALU.subtract)

        with tc.tile_pool(name="st1", bufs=2, space="PSUM") as st_p, \
             tc.tile_pool(name="bc1", bufs=2, space="PSUM") as bc_p, \
             tc.tile_pool(name="sm1", bufs=2) as sm_p, \
             tc.tile_pool(name="sq1", bufs=2) as sq_p:
            for nh in range(NH):
                ln_feat(0, GC, x8g, nh, st_p, bc_p, sm_p, sq_p, g1gb, False)
                ln_feat(GC, CC, x8l, nh, st_p, bc_p, sm_p, sq_p, l1gb, False)

        # ---------------- attention (global PE stream + local DVE stream) ----
        qkT = attn.tile([128, CC, N], BF16, tag="qkT")     # q chunks 0-2, k 3-5
        vpad8 = attn.tile([128, MC, H * (D + 1)], FP8, tag="vpad8")
        ql = attn.tile([128, MC, GD], BF16, tag="ql")
        kl = attn.tile([128, MC, GD], BF16, tag="kl")
        vl = attn.tile([128, MC, GD], BF16, tag="vl")
        o8T = attn.tile([128, 4, N], FP8, tag="o8T")
        nc.gpsimd.memset(o8T[:, GC, :], 0.0)
        km = attn.tile([128, MC, GD], BF16, tag="km")
        kp = attn.tile([128, MC, GD], BF16, tag="kp")
        vm = attn.tile([128, MC, GD], BF16, tag="vm")
        vp = attn.tile([128, MC, GD], BF16, tag="vp")
        o16l = attn.tile([128, MC, GD], BF16, tag="o16l")
        o16Tl = attn.tile([128, GC, N], BF16, tag="o16Tl")

        with tc.tile_pool(name="pq", bufs=2, space="PSUM") as pq_p, \
             tc.tile_pool(name="psc", bufs=2, space="PSUM") as ps_p, \
             tc.tile_pool(name="po", bufs=2, space="PSUM") as po_p, \
             tc.tile_pool(name="esb", bufs=3) as e_p, \
             tc.tile_pool(name="small", bufs=3) as sm2_p, \
             tc.tile_pool(name="lwork", bufs=4) as lw_p:

            # Q^T / K^T
            for nh in range(NH):
                ns = slice(nh * NHW, (nh + 1) * NHW)
                for mo in range(2 * GC):
                    ps = pq_p.tile([128, NHW], F32, tag="pq")
                    for t in range(2):
                        nc.tensor.matmul(ps, wqk[:, 2 * t:2 * t + 2, mo * 128:(mo + 1) * 128],
                                         x8g[:, 2 * t:2 * t + 2, ns],
                                         start=(t == 0), stop=(t == 1), perf_mode=DR)
                    nc.gpsimd.tensor_copy(out=qkT[:, mo, ns], in_=ps)

            # V (token-major, strided into padded layout; pad col = 1.0)
            vview = vpad8.rearrange("p m (h e) -> p m h e", e=D + 1)
            for m in range(MC):
                nc.gpsimd.memset(vview[:, m, :, D], 1.0)
            for m in range(MC):
                ms = slice(m * 128, (m + 1) * 128)
                ps = pq_p.tile([128, NHW], F32, tag="pq")
                psv = ps[:, 0:GD]
                for t in range(2):
                    nc.tensor.matmul(psv, x8g[:, 2 * t:2 * t + 2, ms],
                                     wv[:, 2 * t:2 * t + 2, :],
                                     start=(t == 0), stop=(t == 1), perf_mode=DR)
                nc.gpsimd.tensor_copy(
                    out=vview[:, m, :, 0:D],
                    in_=psv.rearrange("p (h d) -> p h d", d=D))

            # local qkv (all upfront so the local DVE stream can start early)
            for m in range(MC):
                ms = slice(m * 128, (m + 1) * 128)
                for pi in range(3):
                    dst = (ql, kl, vl)[pi]
                    ps_l = pq_p.tile([128, NHW], F32, tag="pq", name="lqkv_ps")
                    psd = ps_l[:, 0:GD]
                    for t in range(2):
                        nc.tensor.matmul(psd, x8l[:, 2 * t:2 * t + 2, ms],
                                         wl[:, 2 * t:2 * t + 2, pi * GD:(pi + 1) * GD],
                                         start=(t == 0), stop=(t == 1), perf_mode=DR)
                    nc.gpsimd.tensor_copy(out=dst[:, m, :], in_=psd)

            # token-shifted copies of local k/v (zero rows at sequence edges)
            for src, dst, d in ((kl, km, -1), (vl, vm, -1), (kl, kp, 1), (vl, vp, 1)):
                if d == -1:
                    nc.sync.dma_start(dst[1:128, :, :], src[0:127, :, :])
                    nc.sync.dma_start(dst[0:1, 1:MC, :], src[127:128, 0:MC - 1, :])
                    nc.sync.dma_start(dst[0:1, 0:1, :], zeros16[0:1, 0:GD])
                else:
                    nc.sync.dma_start(dst[0:127, :, :], src[1:128, :, :])
                    nc.sync.dma_start(dst[127:128, 0:MC - 1, :], src[0:1, 1:MC, :])
                    nc.sync.dma_start(dst[127:128, MC - 1:MC, :], zeros16[0:1, 0:GD])

            # one m-chunk of band attention math (DVE, bf16)
            def emit_local(m):
                qv = ql[:, m].rearrange("p (h d) -> p h d", d=D)
                ed = lw_p.tile([128, 3, H], BF16, tag="ed")
                for di, kk in enumerate((km, kl, kp)):
                    prod = lw_p.tile([128, H, D], BF16, tag="prod")
                    nc.vector.tensor_tensor(prod, qv,
                                            kk[:, m].rearrange("p (h d) -> p h d", d=D),
                                            ALU.mult)
                    with nc.allow_low_precision(reason="bf16 band logits"):
                        nc.vector.reduce_sum(ed[:, di, :], prod, axis=AX.X)
                ee = lw_p.tile([128, 3, H], BF16, tag="ee")
                nc.scalar.activation(ee, ed, AF.Exp, scale=SCALE / (WS * WS))
                if m == 0:
                    nc.vector.memset(ee[0:1, 0, :], 0.0)
                if m == MC - 1:
                    nc.sync.dma_start(ee[127:128, 2, :], zeros16[0:1, 0:H])
                ssum = lw_p.tile([128, H], BF16, tag="ssum")
                rr = lw_p.tile([128, H], BF16, tag="rr")
                nc.vector.tensor_tensor(ssum, ee[:, 0, :], ee[:, 1, :], ALU.add)
                nc.vector.tensor_tensor(ssum, ssum, ee[:, 2, :], ALU.add)
                with nc.allow_low_precision(reason="bf16 softmax weights"):
                    nc.vector.reciprocal(rr, ssum)
                aw = lw_p.tile([128, 3, H], BF16, tag="aw")
                nc.vector.tensor_tensor(aw, ee, rr[:, None, :].to_broadcast((128, 3, H)),
                                        ALU.mult)
                ov = lw_p.tile([128, H, D], BF16, tag="ov")
                t1 = lw_p.tile([128, H, D], BF16, tag="avt")
                nc.vector.tensor_tensor(ov, vm[:, m].rearrange("p (h d) -> p h d", d=D),
                                        aw[:, 0, :, None].to_broadcast((128, H, D)), ALU.mult)
                nc.vector.tensor_tensor(t1, vl[:, m].rearrange("p (h d) -> p h d", d=D),
                                        aw[:, 1, :, None].to_broadcast((128, H, D)), ALU.mult)
                nc.vector.tensor_tensor(ov, ov, t1, ALU.add)
                nc.vector.tensor_tensor(t1, vp[:, m].rearrange("p (h d) -> p h d", d=D),
                                        aw[:, 2, :, None].to_broadcast((128, H, D)), ALU.mult)
                nc.vector.tensor_tensor(o16l[:, m].rearrange("p (h d) -> p h d", d=D),
                                        ov, t1, ALU.add)

            local_ms = list(range(MC))

            # scores (bf16) -> exp (fp8) -> AV (fp8 DoubleRow), local math dripped
            for h in range(H):
                hc, hp = h // 2, (h % 2) * 64
                for nh in range(NH):
                    ns = slice(nh * NHW, (nh + 1) * NHW)
                    po = po_p.tile([D + 1, NHW], F32, tag="po")
                    for mp in range(MC // 2):
                        ps = ps_p.tile([128, 2 * NHW], F32, tag="ps")
                        for half in range(2):
                            m = 2 * mp + half
                            nc.tensor.matmul(ps[:, half * NHW:(half + 1) * NHW],
                                             qkT[hp:hp + 64, GC + hc, m * 128:(m + 1) * 128],
                                             qkT[hp:hp + 64, hc, ns], start=True, stop=True)
                        e8 = e_p.tile([128, 2, NHW], FP8, tag="e8")
                        nc.scalar.activation(e8.rearrange("p a b -> p (a b)"), ps,
                                             AF.Exp, scale=SCALE / (WS * WS))
                        nc.tensor.matmul(po, vpad8[:, 2 * mp:2 * mp + 2,
                                                   h * (D + 1):(h + 1) * (D + 1)],
                                         e8, start=(mp == 0), stop=(mp == MC // 2 - 1),
                                         perf_mode=DR)
                    if local_ms:
                        emit_local(local_ms.pop(0))
                    rcp = sm2_p.tile([1, NHW], BF16, tag="rcp")
                    with nc.allow_low_precision(reason="bf16 matmul operand"):
                        nc.vector.reciprocal(rcp, po[D:D + 1, :])
                    pb = pq_p.tile([128, NHW], F32, tag="pq", name="pbbc")[0:64, :]
                    nc.tensor.matmul(pb, c64, rcp, start=True, stop=True)
                    nc.vector.tensor_tensor(o8T[hp:hp + 64, hc, ns], po[0:D, :], pb, ALU.mult)
            for m in local_ms:
                emit_local(m)

            # transpose O_l to feature-major (PE transposes w/ fp8 identity)
            for m in range(MC):
                pt = ps_p.tile([128, 2 * NHW], F32, tag="ps", name="ptr")
                ptv = pt.bitcast(BF16)[:, 0:GC * 128]
                for c in range(GC):
                    nc.tensor.transpose(ptv[:, c * 128:(c + 1) * 128],
                                        o16l[:, m, c * 128:(c + 1) * 128], ident8)
                nc.gpsimd.tensor_copy(
                    out=o16Tl[:, 0:GC, m * 128:(m + 1) * 128],
                    in_=ptv.rearrange("p (c x) -> p c x", x=128))

            # global proj + residual (PSUM = WS^2 * attn_out)
            for mo in range(GC):
                for nh in range(NH):
                    ns = slice(nh * NHW, (nh + 1) * NHW)
                    ps = pq_p.tile([128, NHW], F32, tag="pq")
                    for t in range(2):
                        nc.tensor.matmul(ps, wpg[:, 2 * t:2 * t + 2, mo * 128:(mo + 1) * 128],
                                         o8T[:, 2 * t:2 * t + 2, ns],
                                         start=(t == 0), stop=(t == 1), perf_mode=DR)
                    if gpb is not None:
                        nc.scalar.activation(ps, ps, AF.Identity,
                                             bias=gpb[:, mo:mo + 1], scale=1.0 / WS)
                        nc.vector.tensor_tensor(xt[:, mo, ns], xt[:, mo, ns], ps, ALU.add)
                    else:
                        nc.vector.scalar_tensor_tensor(xt[:, mo, ns], ps, 1.0 / WS,
                                                       xt[:, mo, ns], ALU.mult, ALU.add)

            # local proj (bf16 moving, fp8 stationary) + residual (PSUM = WS^2 *)
            for mo in range(GC):
                for nh in range(NH):
                    ns = slice(nh * NHW, (nh + 1) * NHW)
                    ps = pq_p.tile([128, NHW], F32, tag="pq")
                    for t in range(GC):
                        nc.tensor.matmul(ps, wpl[:, t, mo * 128:(mo + 1) * 128],
                                         o16Tl[:, t, ns],
                                         start=(t == 0), stop=(t == GC - 1))
                    if lpb is not None:
                        nc.scalar.activation(ps, ps, AF.Identity,
                                             bias=lpb[:, mo:mo + 1], scale=1.0 / (WS * WS))
                        nc.vector.tensor_tensor(xt[:, GC + mo, ns], xt[:, GC + mo, ns],
                                                ps, ALU.add)
                    else:
                        nc.vector.scalar_tensor_tensor(xt[:, GC + mo, ns], ps,
                                                       1.0 / (WS * WS),
                                                       xt[:, GC + mo, ns], ALU.mult, ALU.add)

        attn_scope.__exit__(None, None, None)

        # ---------------- LN2 -> h16 = 4*LN2(x1) -> h8 + hlo8 ----------------
        mlpw = top.enter_context(tc.tile_pool(name="mlpw", bufs=1))
        w1h = mlpw.tile([128, CC, HID], FP8, tag="w1h")
        nc.sync.dma_start(w1h, w1h_d.rearrange("(c p) o -> p c o", p=128))
        w1l = mlpw.tile([128, CC, HID], FP8, tag="w1l")
        nc.sync.dma_start(w1l, w1l_d.rearrange("(c p) o -> p c o", p=128))
        w2h = mlpw.tile([128, JC, C], FP8, tag="w2h")
        nc.sync.dma_start(w2h, w2h_d.rearrange("(c p) o -> p c o", p=128))
        w2l = mlpw.tile([128, JC, C], FP8, tag="w2l")
        nc.sync.dma_start(w2l, w2l_d.rearrange("(c p) o -> p c o", p=128))
        h16 = core.tile([128, CC, N], BF16, tag="h16")
        h8 = core.tile([128, CC, N], FP8, tag="h8")
        hlo8 = core.tile([128, CC, N], FP8, tag="hlo8")
        with tc.tile_pool(name="st2", bufs=2, space="PSUM") as st_p, \
             tc.tile_pool(name="bc2", bufs=2, space="PSUM") as bc_p, \
             tc.tile_pool(name="sm3", bufs=2) as sm_p, \
             tc.tile_pool(name="sq2", bufs=2) as sq_p:
            for nh in range(NH):
                ns = slice(nh * NHW, (nh + 1) * NHW)
                ln_feat(0, CC, h16, nh, st_p, bc_p, sm_p, sq_p, None, True)
                for c in range(CC):
                    nc.gpsimd.tensor_copy(out=h8[:, c, ns], in_=h16[:, c, ns])
                    nc.vector.tensor_tensor(hlo8[:, c, ns], h16[:, c, ns],
                                            h8[:, c, ns], ALU.subtract)

        # ---------------- MLP (fp8 DoubleRow + residual-compensation) --------
        outT = core.tile([128, CC, N], BF16, tag="outT")
        with tc.tile_pool(name="pz", bufs=1, space="PSUM") as pz_p, \
             tc.tile_pool(name="pm", bufs=1, space="PSUM") as pm_p, \
             tc.tile_pool(name="gl", bufs=1) as gl_p:
            for nh in range(NH):
                ns = slice(nh * NHW, (nh + 1) * NHW)
                gl8 = gl_p.tile([128, JC, NHW], FP8, tag="gl8")
                zps = [pz_p.tile([128, NHW], F32, tag=f"z{mo}", name=f"z{mo}")
                       for mo in range(CC)]

                def fc2_group(t2):
                    for mo in range(CC):
                        for wmat, first, last in ((w2h, t2 == 0, False),
                                                  (w2l, False, t2 == JC // 2 - 1)):
                            nc.tensor.matmul(zps[mo],
                                             wmat[:, 2 * t2:2 * t2 + 2,
                                                  mo * 128:(mo + 1) * 128],
                                             gl8[:, 2 * t2:2 * t2 + 2, :],
                                             start=first, stop=last, perf_mode=DR)

                for tp in range(JC // 2):
                    pm = pm_p.tile([128, 2, NHW], F32, tag="pm")
                    for jj in range(2):
                        j = 2 * tp + jj
                        chains = [(w1h, h8), (w1h, hlo8), (w1l, h8)]
                        for ci, (wmat, act) in enumerate(chains):
                            for t in range(GC):
                                nc.tensor.matmul(pm[:, jj, :],
                                                 wmat[:, 2 * t:2 * t + 2,
                                                      j * 128:(j + 1) * 128],
                                                 act[:, 2 * t:2 * t + 2, ns],
                                                 start=(ci == 0 and t == 0),
                                                 stop=(ci == 2 and t == GC - 1),
                                                 perf_mode=DR)
                    if flags["bias_fc1"]:
                        for jj in range(2):
                            nc.scalar.activation(gl8[:, 2 * tp + jj, :], pm[:, jj, :],
                                                 AF.Gelu, bias=fc1b[:, 2 * tp + jj:2 * tp + jj + 1],
                                                 scale=1.0 / (4.0 * WS1))
                    else:
                        nc.scalar.activation(gl8[:, 2 * tp:2 * tp + 2, :], pm, AF.Gelu,
                                             bias=0.0, scale=1.0 / (4.0 * WS1))
                    if tp > 0:
                        fc2_group(tp - 1)
                fc2_group(JC // 2 - 1)

                for mo in range(CC):
                    if fc2b is not None:
                        nc.scalar.activation(zps[mo], zps[mo], AF.Identity,
                                             bias=fc2b[:, mo:mo + 1], scale=1.0 / WS2)
                        nc.vector.tensor_tensor(outT[:, mo, ns], xt[:, mo, ns],
                                                zps[mo], ALU.add)
                    else:
                        nc.vector.scalar_tensor_tensor(outT[:, mo, ns], zps[mo],
                                                       1.0 / WS2,
                                                       xt[:, mo, ns], ALU.mult, ALU.add)
                nc.sync.dma_start(
                    out_d.rearrange("(c p) n -> p c n", p=128)[:, :, ns],
                    outT[:, :, ns])

    nc.compile()
    return nc


def _prep_weights(inp):
    """Host-side: fold LN2 affine into fc1, quantize weights to fp8 with
    residual-compensation pairs for the MLP."""
    def q8(w, scale):
        return np.clip(w.astype(np.float64) * scale, -240.0, 240.0).astype(NP_FP8)

    def q8_pair(w, scale):
        hi = q8(w, scale)
        resid = w.astype(np.float64) * scale - hi.astype(np.float64)
        lo = np.clip(resid, -240.0, 240.0).astype(NP_FP8)
        return hi, lo

    def pad_rows(w, rows):
        out = np.zeros((rows, w.shape[1]), np.float64)
        out[:w.shape[0]] = w
        return out

    gqkv = inp["g_qkv_w"]
    lqkv = inp["l_qkv_w"]
    fc1_w = inp["fc1_w"] * inp["ln2_g"][:, None]
    fc1_bias = inp["fc1_b"].astype(np.float64) + \
        inp["ln2_b"].astype(np.float64) @ inp["fc1_w"].astype(np.float64)
    w1h, w1l = q8_pair(fc1_w, WS1)
    w2h, w2l = q8_pair(inp["fc2_w"], WS2)
    return {
        "wqk8": q8(pad_rows(gqkv[:, :2 * GD], 512), WS),
        "wv8": q8(pad_rows(gqkv[:, 2 * GD:], 512), WS),
        "wl8": q8(pad_rows(lqkv, 512), WS),
        "wpg8": q8(pad_rows(inp["g_proj_w"], 512), WS),
        "wpl8": q8(pad_rows(inp["l_proj_w"], 512), WS),
        "w1h8": w1h, "w1l8": w1l, "w2h8": w2h, "w2l8": w2l,
        "fc1bias": fc1_bias.astype(np.float32),
    }


_NC_CACHE = {}


def kernel(**inputs):
    inp = {k: np.ascontiguousarray(np.asarray(v), dtype=np.float32) for k, v in inputs.items()}
    wmap = _prep_weights(inp)
    flags = {
        "gb1g": not (np.all(inp["ln1_g"] == 1.0) and np.all(inp["ln1_b"] == 0.0)),
        "gb1l": not (np.all(inp["ln1l_g"] == 1.0) and np.all(inp["ln1l_b"] == 0.0)),
        "bias_gproj": bool(np.any(inp["g_proj_b"] != 0.0)),
        "bias_lproj": bool(np.any(inp["l_proj_b"] != 0.0)),
        "bias_fc1": bool(np.any(wmap["fc1bias"] != 0.0)),
        "bias_fc2": bool(np.any(inp["fc2_b"] != 0.0)),
    }
    key = tuple(sorted(flags.items()))
    nc = _NC_CACHE.get(key)
    if nc is None:
        nc = _build(flags)
        _NC_CACHE[key] = nc
    if flags["bias_gproj"]:
        wmap["gpb"] = inp["g_proj_b"]
    if flags["bias_lproj"]:
        wmap["lpb"] = inp["l_proj_b"]
    if flags["bias_fc2"]:
        wmap["fc2bias"] = inp["fc2_b"]
    if flags["gb1g"]:
        wmap["ln1gb"] = np.stack([inp["ln1_g"], inp["ln1_b"]])
    if flags["gb1l"]:
        wmap["ln1lgb"] = np.stack([inp["ln1l_g"], inp["ln1l_b"]])

    x = inp["x"]
    in_maps = [dict(wmap, xT=np.ascontiguousarray(x[b].T).astype(NP_BF16))
               for b in range(B)]
    trace = os.environ.get("BASS_KERNEL_TRACE", "") == "1"
    res = run_bass_kernel_spmd(nc, in_maps, core_ids=list(range(B)),
                               trace=trace, trace_cores=[0] if trace else None)
    if trace:
        print(f"HW exec time: {res.exec_time_ns} ns")
        if res.instructions_and_trace:
            print("trace path:", res.instructions_and_trace[1])
    return np.stack([np.asarray(res.results[b]["outT"]).astype(np.float32).T
                     for b in range(B)])
